# revision 1
# baseline (speedup 1.0000x reference)
"""ClustGeoNodeEncoder on 8 Trainium2 NeuronCores (Bass/Tile).

Pipeline (data-parallel over voxels, per the sharding hint):
  pass 1  per-core segment-sum of 10 moments [1,x,y,z,xx,xy,xz,yy,yz,zz]
          into [128(lo) x 32(hi) x 10] PSUM via fp32r matmuls:
            stationary = one-hot(lo = id & 127)      [128 voxels, 128]
            moving     = (hi(id) == hi) * feat       [128 voxels, 32*10]
          AllReduce partials across the 8 cores.
  phase C closed-form symmetric 3x3 eigensolve per cluster on [128,32]
          tiles (trig method; B = cov / w2 exactly since delta == 0;
          principal axis via Cayley-Hamilton (A-w0)(A-w1)).
  pass 2  dma_gather of per-cluster (center, v0hat) rows per voxel,
          per-voxel val = x0 * ||xc - x0 v0||,
  pass 3  segment-sum of val via plain-fp32 matmul (sign-critical),
          AllReduce, then sign fixup + output assembly [4096, 16].

Self-contained: hardcodes shapes from the problem spec.
"""
import numpy as np

import concourse.bass as bass
import concourse.tile as tile
from concourse import bacc, mybir
from concourse import bass_utils

P = 128
C = 4096
NHI = 32
F = 10
NCORES = 8
N_FULL = 4_000_000

f32 = mybir.dt.float32
f32r = mybir.dt.float32r
i16 = mybir.dt.int16
AO = mybir.AluOpType
AF = mybir.ActivationFunctionType

TINY = 1e-30


def build(V=3968, T1=64, MB=16, X2=128, n_cores=NCORES, stop_after="full"):
    """Build + compile the SPMD program. V = voxel slots per partition."""
    assert V % T1 == 0 and V % X2 == 0 and T1 % MB == 0
    NCH1 = V // T1
    NCH2 = V // X2

    nc = bacc.Bacc("TRN2", target_bir_lowering=False, debug=False,
                   enable_asserts=False, num_devices=n_cores)
    data_d = nc.dram_tensor("data", [P, V, 5], f32, kind="ExternalInput")
    ids_d = nc.dram_tensor("ids", [P, V], i16, kind="ExternalInput")
    io128_d = nc.dram_tensor("iota128", [128], f32, kind="ExternalInput")
    io32_d = nc.dram_tensor("iota32", [32], f32, kind="ExternalInput")
    out_d = nc.dram_tensor("out", [C, 16], f32, kind="ExternalOutput")

    groups = [list(range(n_cores))]

    from contextlib import ExitStack
    with tile.TileContext(nc) as tc, ExitStack() as stk:
        cpool = stk.enter_context(tc.tile_pool(name="consts", bufs=1))
        dram = stk.enter_context(tc.tile_pool(name="dram", bufs=1, space="DRAM"))
        ppool = stk.enter_context(tc.tile_pool(name="psum", bufs=1, space="PSUM"))
        spool = stk.enter_context(tc.tile_pool(name="small", bufs=1))

        iota128 = cpool.tile([P, 128], f32)
        iota32 = cpool.tile([P, NHI], f32)
        nc.sync.dma_start(iota128[:], io128_d.ap()[None, :].to_broadcast([P, 128]))
        nc.sync.dma_start(iota32[:], io32_d.ap()[None, :].to_broadcast([P, NHI]))

        # ---------------- pass 1: moment segment-sums ----------------
        ps1 = ppool.tile([P, NHI * F], f32, space="PSUM")
        with tc.tile_pool(name="p1", bufs=2) as p1, \
             tc.tile_pool(name="p1mf", bufs=2) as p1mf, \
             tc.tile_pool(name="p1oh", bufs=4) as p1oh:
            for c in range(NCH1):
                sl = slice(c * T1, (c + 1) * T1)
                dat = p1.tile([P, T1, 5], f32, tag="dat")
                idc = p1.tile([P, T1], i16, tag="idc")
                nc.sync.dma_start(dat[:], data_d.ap()[:, sl, :])
                nc.sync.dma_start(idc[:], ids_d.ap()[:, sl])

                hi_i = p1.tile([P, T1], i16, tag="hii")
                lo_i = p1.tile([P, T1], i16, tag="loi")
                nc.vector.tensor_scalar(out=hi_i[:], in0=idc[:], scalar1=7,
                                        scalar2=None, op0=AO.logical_shift_right)
                nc.vector.tensor_scalar(out=lo_i[:], in0=idc[:], scalar1=127,
                                        scalar2=None, op0=AO.bitwise_and)
                hi_f = p1.tile([P, T1], f32, tag="hif")
                lo_f = p1.tile([P, T1], f32, tag="lof")
                nc.vector.tensor_copy(out=hi_f[:], in_=hi_i[:])
                nc.vector.tensor_copy(out=lo_f[:], in_=lo_i[:])

                feat = p1.tile([P, T1, F], f32, tag="feat")
                nc.vector.memset(feat[:], 1.0)
                nc.vector.tensor_copy(out=feat[:, :, 1:4], in_=dat[:, :, 0:3])
                nc.vector.tensor_tensor(
                    out=feat[:, :, 4:7],
                    in0=dat[:, :, 0:1].to_broadcast([P, T1, 3]),
                    in1=dat[:, :, 0:3], op=AO.mult)
                nc.vector.tensor_tensor(
                    out=feat[:, :, 7:9],
                    in0=dat[:, :, 1:2].to_broadcast([P, T1, 2]),
                    in1=dat[:, :, 1:3], op=AO.mult)
                nc.vector.tensor_tensor(
                    out=feat[:, :, 9:10],
                    in0=dat[:, :, 2:3], in1=dat[:, :, 2:3], op=AO.mult)

                for t in range(T1):
                    mf = p1mf.tile([P, NHI, F], f32r, tag="mf")
                    nc.vector.scalar_tensor_tensor(
                        out=mf[:],
                        in0=iota32[:].unsqueeze(2).to_broadcast([P, NHI, F]),
                        scalar=hi_f[:, t:t + 1],
                        in1=feat[:, t].unsqueeze(1).to_broadcast([P, NHI, F]),
                        op0=AO.is_equal, op1=AO.mult)
                    oh = p1oh.tile([P, 128], f32r, tag="oh")
                    nc.vector.tensor_scalar(
                        out=oh[:], in0=iota128[:],
                        scalar1=lo_f[:, t:t + 1], scalar2=None,
                        op0=AO.is_equal)
                    nc.tensor.matmul(
                        out=ps1[:], lhsT=oh[:],
                        rhs=mf[:].rearrange("p a b -> p (a b)"),
                        start=(c == 0 and t == 0),
                        stop=(c == NCH1 - 1 and t == T1 - 1))

        # ---------------- AllReduce moments ----------------
        S = spool.tile([P, NHI, F], f32)
        nc.vector.tensor_copy(out=S[:].rearrange("p a b -> p (a b)"), in_=ps1[:])
        cc1_in = dram.tile([P, NHI * F], f32)
        cc1_out = dram.tile([P, NHI * F], f32, addr_space="Shared")
        nc.sync.dma_start(cc1_in[:], S[:].rearrange("p a b -> p (a b)"))
        nc.gpsimd.collective_compute(
            "AllReduce", AO.add, replica_groups=groups,
            ins=[cc1_in[:]], outs=[cc1_out[:]])
        nc.sync.dma_start(S[:].rearrange("p a b -> p (a b)"), cc1_out[:])

        # ---------------- phase C: per-cluster eigensolve ----------------
        def new(name):
            return spool.tile([P, NHI], f32, name=name)

        def tt(out, a, b, op):
            nc.vector.tensor_tensor(out=out[:], in0=a[:], in1=b[:], op=op)

        def ts_(out, a, s1, op, s2=None, op1=None):
            kw = {}
            if op1 is not None:
                kw = dict(op1=op1)
            nc.vector.tensor_scalar(out=out[:], in0=a[:], scalar1=s1, scalar2=s2,
                                    op0=op, **kw)

        dtmp1 = new("dtmp1"); dtmp2 = new("dtmp2")

        def recip(out, den):
            # out = 1/den with one Newton refinement (den must be nonzero)
            nc.vector.reciprocal(out=dtmp1[:], in_=den[:])
            tt(dtmp2, den, dtmp1, AO.mult)
            ts_(dtmp2, dtmp2, -1.0, AO.mult, s2=2.0, op1=AO.add)
            tt(out, dtmp1, dtmp2, AO.mult)

        cnt = new("cnt"); safe = new("safe")
        nc.vector.tensor_copy(out=cnt[:], in_=S[:, :, 0])
        ts_(safe, cnt, 1.0, AO.max)
        rsafe = new("rsafe")
        recip(rsafe, safe)

        ctr = [new(f"ctr{k}") for k in range(3)]
        for k in range(3):
            nc.vector.tensor_tensor(out=ctr[k][:], in0=S[:, :, 1 + k],
                                    in1=rsafe[:], op=AO.mult)

        # cov_ij = q_ij - cnt*ctr_i*ctr_j   (order xx,xy,xz,yy,yz,zz)
        cc = [new(f"cc{k}") for k in range(3)]
        for k in range(3):
            tt(cc[k], cnt, ctr[k], AO.mult)
        pairs = [(0, 0), (0, 1), (0, 2), (1, 1), (1, 2), (2, 2)]
        cov = [new(f"cov{k}") for k in range(6)]
        tmp = new("tmp"); tmp2 = new("tmp2"); tmp3 = new("tmp3")
        for k, (i, j) in enumerate(pairs):
            tt(tmp, ctr[i], cc[j], AO.mult)
            nc.vector.tensor_tensor(out=cov[k][:], in0=S[:, :, 4 + k],
                                    in1=tmp[:], op=AO.subtract)
        XX, XY, XZ, YY, YZ, ZZ = range(6)

        # eigenvalues: trig method
        qm = new("qm")
        tt(qm, cov[XX], cov[YY], AO.add)
        tt(qm, qm, cov[ZZ], AO.add)
        ts_(qm, qm, 1.0 / 3.0, AO.mult)
        aq = [new(f"aq{k}") for k in range(3)]
        tt(aq[0], cov[XX], qm, AO.subtract)
        tt(aq[1], cov[YY], qm, AO.subtract)
        tt(aq[2], cov[ZZ], qm, AO.subtract)
        p2 = new("p2")
        tt(p2, aq[0], aq[0], AO.mult)
        tt(tmp, aq[1], aq[1], AO.mult); tt(p2, p2, tmp, AO.add)
        tt(tmp, aq[2], aq[2], AO.mult); tt(p2, p2, tmp, AO.add)
        for k in (XY, XZ, YZ):
            tt(tmp, cov[k], cov[k], AO.mult)
            ts_(tmp, tmp, 2.0, AO.mult)
            tt(p2, p2, tmp, AO.add)
        pp = new("pp")
        ts_(tmp, p2, 1.0 / 6.0, AO.mult)
        nc.scalar.sqrt(pp[:], tmp[:])
        psafe = new("psafe"); rpsafe = new("rpsafe")
        ts_(psafe, pp, TINY, AO.max)
        recip(rpsafe, psafe)

        # normalized B matrix entries b_k = (cov - qm*delta)/p
        bn = [new(f"bn{k}") for k in range(6)]
        for k, (i, j) in enumerate(pairs):
            src = aq[i] if i == j else cov[k]
            if i == j:
                src = aq[{0: 0, 3: 1, 5: 2}[k]]
            nc.vector.tensor_tensor(out=bn[k][:], in0=src[:], in1=rpsafe[:],
                                    op=AO.mult)
        # r = det(bn)/2, clamped to [-1, 1]
        det = new("det")
        tt(tmp, bn[YY], bn[ZZ], AO.mult)
        tt(tmp2, bn[YZ], bn[YZ], AO.mult)
        tt(tmp, tmp, tmp2, AO.subtract)
        tt(det, bn[XX], tmp, AO.mult)
        tt(tmp, bn[XY], bn[ZZ], AO.mult)
        tt(tmp2, bn[YZ], bn[XZ], AO.mult)
        tt(tmp, tmp, tmp2, AO.subtract)
        tt(tmp, tmp, bn[XY], AO.mult)
        tt(det, det, tmp, AO.subtract)
        tt(tmp, bn[XY], bn[YZ], AO.mult)
        tt(tmp2, bn[YY], bn[XZ], AO.mult)
        tt(tmp, tmp, tmp2, AO.subtract)
        tt(tmp, tmp, bn[XZ], AO.mult)
        tt(det, det, tmp, AO.add)
        r = new("r")
        ts_(r, det, 0.5, AO.mult, s2=1.0, op1=AO.min)
        ts_(r, r, -1.0, AO.max)

        # phi = acos(r)/3 via t = atan(sqrt(1-r^2)/(1+|r|)) in [0, pi/4]:
        #   acos(r) = 2t for r >= 0, pi - 2t for r < 0
        omr = new("omr"); opr = new("opr"); sig = new("sig"); absr = new("absr")
        ts_(omr, r, -1.0, AO.mult, s2=1.0, op1=AO.add)      # 1 - r
        ts_(opr, r, 1.0, AO.add)                            # 1 + r
        tt(tmp, omr, opr, AO.mult)
        nc.scalar.sqrt(sig[:], tmp[:])
        nc.scalar.activation(absr[:], r[:], AF.Abs)
        ts_(tmp, absr, 1.0, AO.add)                          # 1 + |r| in [1,2]
        recip(tmp3, tmp)
        tt(tmp2, sig, tmp3, AO.mult)                         # in [0, 1]
        phi = new("phi"); rneg = new("rneg")
        nc.scalar.activation(phi[:], tmp2[:], AF.Arctan)
        ts_(phi, phi, 2.0 / 3.0, AO.mult)                    # acos(|r|)/3
        ts_(rneg, r, 0.0, AO.is_lt)
        # phi = (1-2*rneg)*phi + rneg*pi/3
        ts_(tmp, rneg, -2.0, AO.mult, s2=1.0, op1=AO.add)
        tt(phi, phi, tmp, AO.mult)
        ts_(tmp, rneg, float(np.pi / 3.0), AO.mult)
        tt(phi, phi, tmp, AO.add)

        # w2 = qm + 2 p cos(phi);  w0 = qm + 2 p sin(-pi/6 - phi)
        w0 = new("w0"); w1 = new("w1"); w2 = new("w2")
        ts_(tmp, phi, -1.0, AO.mult, s2=float(np.pi / 2), op1=AO.add)
        nc.scalar.activation(tmp2[:], tmp[:], AF.Sin)
        tt(tmp2, tmp2, pp, AO.mult)
        ts_(tmp2, tmp2, 2.0, AO.mult)
        tt(w2, qm, tmp2, AO.add)
        ts_(tmp, phi, -1.0, AO.mult, s2=float(-np.pi / 6), op1=AO.add)
        nc.scalar.activation(tmp2[:], tmp[:], AF.Sin)
        tt(tmp2, tmp2, pp, AO.mult)
        ts_(tmp2, tmp2, 2.0, AO.mult)
        tt(w0, qm, tmp2, AO.add)
        ts_(tmp, qm, 3.0, AO.mult)
        tt(tmp, tmp, w0, AO.subtract)
        tt(w1, tmp, w2, AO.subtract)

        # dirwt = (w2 == 0) ? 0 : 1 - w1/w2
        w2z = new("w2z"); dirwt = new("dirwt")
        ts_(w2z, w2, 0.0, AO.is_equal)
        ts_(tmp, w2, TINY, AO.max)
        recip(tmp3, tmp)
        tt(tmp2, w1, tmp3, AO.mult)
        ts_(tmp2, tmp2, -1.0, AO.mult, s2=1.0, op1=AO.add)  # 1 - w1/w2
        ts_(tmp, w2z, -1.0, AO.mult, s2=1.0, op1=AO.add)    # 1 - w2z
        tt(dirwt, tmp2, tmp, AO.mult)

        # B = cov / (w2 == 0 ? 1 : w2)
        denb = new("denb"); rdenb = new("rdenb")
        tt(denb, w2, w2z, AO.add)
        recip(rdenb, denb)
        Bk = [new(f"B{k}") for k in range(6)]
        for k in range(6):
            tt(Bk[k], cov[k], rdenb, AO.mult)

        # principal axis: M = (A - w0 I)(A - w1 I); columns span v2
        d0 = [new(f"d0{k}") for k in range(3)]
        d1 = [new(f"d1{k}") for k in range(3)]
        for k, dk in enumerate((XX, YY, ZZ)):
            tt(d0[k], cov[dk], w0, AO.subtract)
            tt(d1[k], cov[dk], w1, AO.subtract)
        # rows of A0: [d0[0], XY, XZ; XY, d0[1], YZ; XZ, YZ, d0[2]]
        A0 = [[d0[0], cov[XY], cov[XZ]],
              [cov[XY], d0[1], cov[YZ]],
              [cov[XZ], cov[YZ], d0[2]]]
        A1 = [[d1[0], cov[XY], cov[XZ]],
              [cov[XY], d1[1], cov[YZ]],
              [cov[XZ], cov[YZ], d1[2]]]
        M = [[new(f"M{i}{j}") for j in range(3)] for i in range(3)]
        for i in range(3):
            for j in range(3):
                tt(M[i][j], A0[i][0], A1[0][j], AO.mult)
                tt(tmp, A0[i][1], A1[1][j], AO.mult)
                tt(M[i][j], M[i][j], tmp, AO.add)
                tt(tmp, A0[i][2], A1[2][j], AO.mult)
                tt(M[i][j], M[i][j], tmp, AO.add)
        nrm = [new(f"nrm{j}") for j in range(3)]
        for j in range(3):
            tt(nrm[j], M[0][j], M[0][j], AO.mult)
            tt(tmp, M[1][j], M[1][j], AO.mult)
            tt(nrm[j], nrm[j], tmp, AO.add)
            tt(tmp, M[2][j], M[2][j], AO.mult)
            tt(nrm[j], nrm[j], tmp, AO.add)
        vbest = [new(f"vb{i}") for i in range(3)]
        nbest = new("nbest")
        mask = spool.tile([P, NHI], mybir.dt.int32, name="mask")
        tt(mask, nrm[1], nrm[0], AO.is_gt)
        for i in range(3):
            nc.vector.select(vbest[i][:], mask[:], M[i][1][:], M[i][0][:])
        nc.vector.select(nbest[:], mask[:], nrm[1][:], nrm[0][:])
        tt(mask, nrm[2], nbest, AO.is_gt)
        for i in range(3):
            nc.vector.select(vbest[i][:], mask[:], M[i][2][:], vbest[i][:])
        nc.vector.select(nbest[:], mask[:], nrm[2][:], nbest[:])
        vhat = [new(f"vh{i}") for i in range(3)]
        nc.scalar.sqrt(tmp[:], nbest[:])
        ts_(tmp, tmp, TINY, AO.max)
        recip(tmp3, tmp)
        for i in range(3):
            tt(vhat[i], vbest[i], tmp3, AO.mult)

        small = new("small"); notsmall = new("notsmall")
        ts_(small, cnt, 2.0, AO.is_lt)
        ts_(notsmall, small, -1.0, AO.mult, s2=1.0, op1=AO.add)

        # gather table rows: [ctr_x, ctr_y, ctr_z, vh_x, vh_y, vh_z]
        table_d = dram.tile([C, 16], f32)
        G = spool.tile([P, NHI, 6], f32)
        for k in range(3):
            nc.vector.tensor_copy(out=G[:, :, k], in_=ctr[k][:])
            nc.vector.tensor_copy(out=G[:, :, 3 + k], in_=vhat[k][:])
        # DRAM row c = hi*128 + lo  ->  partitions are lo, free dim hi
        nc.sync.dma_start(
            table_d[:].rearrange("(a l) e -> l a e", l=P)[:, :NHI, 0:6], G[:])

        if stop_after == "pc":
            OUTD = spool.tile([P, NHI, 16], f32)
            nc.vector.memset(OUTD[:], 0.0)
            for k in range(3):
                nc.vector.tensor_copy(out=OUTD[:, :, k], in_=ctr[k][:])
                nc.vector.tensor_copy(out=OUTD[:, :, 12 + k], in_=vhat[k][:])
            nc.vector.tensor_copy(out=OUTD[:, :, 15], in_=cnt[:])
            nc.vector.tensor_copy(out=OUTD[:, :, 3], in_=dirwt[:])
            nc.vector.tensor_copy(out=OUTD[:, :, 4], in_=w1[:])
            nc.vector.tensor_copy(out=OUTD[:, :, 5], in_=w2[:])
            nc.sync.dma_start(
                out_d.ap().rearrange("(a l) e -> l a e", l=P), OUTD[:])

        if stop_after != "pc":
            # ---------------- pass 2/3: sc segment-sum ----------------
            ps_sc = ppool.tile([P, NHI], f32, space="PSUM")
            with tc.tile_pool(name="p2", bufs=2) as p2, \
                 tc.tile_pool(name="p2g", bufs=2) as p2g, \
                 tc.tile_pool(name="p2oh", bufs=4) as p2oh:
                for c in range(NCH2):
                    sl = slice(c * X2, (c + 1) * X2)
                    dat = p2.tile([P, X2, 5], f32, tag="dat2")
                    idc = p2.tile([P, X2], i16, tag="idc2")
                    nc.sync.dma_start(dat[:], data_d.ap()[:, sl, :])
                    nc.sync.dma_start(idc[:], ids_d.ap()[:, sl])

                    idg = p2g.tile([P, X2], mybir.dt.int32, tag="idg")
                    nc.vector.tensor_scalar(out=idg[:], in0=idc[:],
                                            scalar1=C - 1, scalar2=None,
                                            op0=AO.min)
                    gat = p2g.tile([P, X2, 16], f32, tag="gat")
                    if "nogather" in stop_after:
                        nc.vector.memset(gat[:, :, 0:8], 0.125)
                    else:
                        # HW supports one offset per partition per indirect
                        # DMA: gather one 64B row per voxel column.
                        for t in range(X2):
                            nc.gpsimd.indirect_dma_start(
                                out=gat[:, t, :], out_offset=None,
                                in_=table_d[:],
                                in_offset=bass.IndirectOffsetOnAxis(
                                    ap=idg[:, t:t + 1], axis=0))

                    hi_i = p2.tile([P, X2], i16, tag="hii2")
                    hi_f = p2.tile([P, X2], f32, tag="hif2")
                    lo_i = p2.tile([P, X2], i16, tag="loi2")
                    lo_f = p2.tile([P, X2], f32, tag="lof2")
                    nc.vector.tensor_scalar(out=hi_i[:], in0=idc[:], scalar1=7,
                                            scalar2=None, op0=AO.logical_shift_right)
                    nc.vector.tensor_scalar(out=lo_i[:], in0=idc[:], scalar1=127,
                                            scalar2=None, op0=AO.bitwise_and)
                    nc.vector.tensor_copy(out=hi_f[:], in_=hi_i[:])
                    nc.vector.tensor_copy(out=lo_f[:], in_=lo_i[:])

                    xc = p2.tile([P, X2, 3], f32, tag="xc")
                    nc.vector.tensor_tensor(out=xc[:], in0=dat[:, :, 0:3],
                                            in1=gat[:, :, 0:3], op=AO.subtract)
                    prod = p2.tile([P, X2, 3], f32, tag="prod")
                    nc.vector.tensor_tensor(out=prod[:], in0=xc[:],
                                            in1=gat[:, :, 3:6], op=AO.mult)
                    x0 = p2.tile([P, X2], f32, tag="x0")
                    nc.vector.tensor_reduce(out=x0[:], in_=prod[:],
                                            axis=mybir.AxisListType.X, op=AO.add)
                    nc.vector.tensor_tensor(out=prod[:], in0=xc[:], in1=xc[:],
                                            op=AO.mult)
                    nsq = p2.tile([P, X2], f32, tag="nsq")
                    nc.vector.tensor_reduce(out=nsq[:], in_=prod[:],
                                            axis=mybir.AxisListType.X, op=AO.add)
                    val = p2.tile([P, X2], f32, tag="val")
                    # val = x0 * sqrt(max(nsq - x0^2, 0))
                    nc.vector.tensor_tensor(out=val[:], in0=x0[:], in1=x0[:],
                                            op=AO.mult)
                    nc.vector.tensor_tensor(out=val[:], in0=nsq[:], in1=val[:],
                                            op=AO.subtract)
                    nc.vector.tensor_scalar(out=val[:], in0=val[:], scalar1=0.0,
                                            scalar2=None, op0=AO.max)
                    nc.scalar.sqrt(val[:], val[:])
                    nc.vector.tensor_tensor(out=val[:], in0=val[:], in1=x0[:],
                                            op=AO.mult)

                    for t in range(X2):
                        oh = p2oh.tile([P, 128], f32, tag="oh3")
                        nc.vector.tensor_scalar(
                            out=oh[:], in0=iota128[:],
                            scalar1=lo_f[:, t:t + 1], scalar2=None,
                            op0=AO.is_equal)
                        mf1 = p2oh.tile([P, NHI], f32, tag="mf1")
                        nc.vector.scalar_tensor_tensor(
                            out=mf1[:], in0=iota32[:], scalar=hi_f[:, t:t + 1],
                            in1=val[:, t:t + 1].to_broadcast([P, NHI]),
                            op0=AO.is_equal, op1=AO.mult)
                        nc.tensor.matmul(
                            out=ps_sc[:], lhsT=oh[:], rhs=mf1[:],
                            start=(c == 0 and t == 0),
                            stop=(c == NCH2 - 1 and t == X2 - 1))

            scl = spool.tile([P, NHI], f32)
            if "nomm3" in stop_after:
                nc.vector.memset(scl[:], 1.0)
            else:
                nc.vector.tensor_copy(out=scl[:], in_=ps_sc[:])
            sc = spool.tile([P, NHI], f32)
            if "nocc2" in stop_after:
                nc.vector.tensor_copy(out=sc[:], in_=scl[:])
            else:
                cc2_in = dram.tile([P, NHI], f32)
                cc2_out = dram.tile([P, NHI], f32, addr_space="Shared")
                nc.sync.dma_start(cc2_in[:], scl[:])
                nc.gpsimd.collective_compute(
                    "AllReduce", AO.add, replica_groups=groups,
                    ins=[cc2_in[:]], outs=[cc2_out[:]])
                nc.sync.dma_start(sc[:], cc2_out[:])

            # ---------------- phase E: assemble output ----------------
            flip = new("flip"); scale = new("scale")
            ts_(flip, sc, 0.0, AO.is_lt)
            ts_(flip, flip, -2.0, AO.mult, s2=1.0, op1=AO.add)  # 1 - 2*(sc<0)
            tt(scale, dirwt, flip, AO.mult)
            tt(scale, scale, notsmall, AO.mult)

            OUT = spool.tile([P, NHI, 16], f32)
            for k in range(3):
                nc.vector.tensor_copy(out=OUT[:, :, k], in_=ctr[k][:])
            bidx = [XX, XY, XZ, XY, YY, YZ, XZ, YZ, ZZ]
            for k in range(9):
                tt(tmp, Bk[bidx[k]], notsmall, AO.mult)
                nc.vector.tensor_copy(out=OUT[:, :, 3 + k], in_=tmp[:])
            for k in range(3):
                tt(tmp, vhat[k], scale, AO.mult)
                nc.vector.tensor_copy(out=OUT[:, :, 12 + k], in_=tmp[:])
            nc.vector.tensor_copy(out=OUT[:, :, 15], in_=cnt[:])
            nc.sync.dma_start(
                out_d.ap().rearrange("(a l) e -> l a e", l=P), OUT[:])
    nc.compile()
    return nc


_CACHE = {}


def _get(V, T1, MB, X2, n_cores, stop_after="full"):
    key = (V, T1, MB, X2, n_cores, stop_after)
    if key not in _CACHE:
        _CACHE[key] = build(V, T1, MB, X2, n_cores, stop_after)
    return _CACHE[key]


def run(data, clust_ids, V=3968, T1=64, MB=16, X2=128, n_cores=NCORES,
        stop_after="full"):
    """data [N,>=3] f32, clust_ids [N] int -> [4096, 16] f32."""
    n = data.shape[0]
    per = n // n_cores
    assert per * n_cores == n and per <= P * V
    data = np.ascontiguousarray(np.asarray(data[:, :5], dtype=np.float32))
    ids = np.asarray(clust_ids).astype(np.int16)
    io128 = np.arange(128, dtype=np.float32)
    io32 = np.arange(NHI, dtype=np.float32)
    in_maps = []
    for k in range(n_cores):
        dpad = np.zeros((P * V, 5), np.float32)
        dpad[:per] = data[k * per:(k + 1) * per]
        ipad = np.full((P * V,), C, np.int16)
        ipad[:per] = ids[k * per:(k + 1) * per]
        in_maps.append({"data": dpad.reshape(P, V, 5),
                        "ids": ipad.reshape(P, V),
                        "iota128": io128, "iota32": io32})
    nc = _get(V, T1, MB, X2, n_cores, stop_after)
    res = bass_utils.run_bass_kernel_spmd(nc, in_maps,
                                          core_ids=list(range(n_cores)))
    return res.results[0]["out"], res


def kernel(data, clust_ids, num_clusters=C):
    out, _ = run(np.asarray(data), np.asarray(clust_ids))
    return out



# revision 11
# speedup vs baseline: 1.3009x; 1.3009x over previous
"""ClustGeoNodeEncoder on 8 Trainium2 NeuronCores (Bass/Tile) — v2.

Pipeline (data-parallel over voxels):
  pass 1  per-core segment-sum of 10 moments [1,x,y,z,xx,xy,xz,yy,yz,zz]
          into PSUM [128(lo) x 32(hi) x 10] via bf16 matmuls:
            stationary = one-hot(lo = id & 127)   (TS is_equal @4x, DVE)
            moving     = mask(hi) * feat          (chunked TT, split DVE/Pool)
          AllReduce partials across the 8 cores.
  phase C closed-form symmetric 3x3 eigensolve per cluster on [128,32]
          tiles (trig method); writes a [4096,8] bf16 gather table
          (center, v0hat) to DRAM.
  pass 2  ONE batched indirect DMA gather per chunk (offsets [128,G])
          of per-cluster rows, then per-voxel val = x0 * ||xc - x0 v0||.
  pass 3  segment-sum of val via bf16 matmul into [128(lo) x 32(hi)],
          AllReduce, sign fixup + output assembly [4096, 16].

Self-contained: hardcodes shapes from the problem spec.
"""
import numpy as np

import concourse.bass as bass
import concourse.tile as tile
from concourse import bacc, mybir
from concourse import bass_utils

P = 128
C = 4096
NHI = 32
F = 10
NCORES = 8
N_FULL = 4_000_000

f32 = mybir.dt.float32
bf16 = mybir.dt.bfloat16
i32 = mybir.dt.int32
AO = mybir.AluOpType
AF = mybir.ActivationFunctionType

TINY = 1e-30


def build(V=3968, G=64, n_cores=NCORES, stop_after="full", mf_dve=12):
    """V = voxel slots per partition, G = columns per chunk.

    mf_dve: how many of the 32 hi-planes of mf the DVE builds (rest Pool).
    """
    assert V % G == 0
    NCH = V // G

    nc = bacc.Bacc("TRN2", target_bir_lowering=False, debug=False,
                   enable_asserts=False, num_devices=n_cores)
    xyz_d = nc.dram_tensor("xyz", [P, V, 3], bf16, kind="ExternalInput")
    lo_d = nc.dram_tensor("lo", [P, V], bf16, kind="ExternalInput")
    hi_d = nc.dram_tensor("hi", [P, V], bf16, kind="ExternalInput")
    idg_d = nc.dram_tensor("idg", [P, V], i32, kind="ExternalInput")
    io128_d = nc.dram_tensor("iota128", [128], bf16, kind="ExternalInput")
    io32_d = nc.dram_tensor("iota32", [32], bf16, kind="ExternalInput")
    out_d = nc.dram_tensor("out", [C, 16], f32, kind="ExternalOutput")

    groups = [list(range(n_cores))]

    from contextlib import ExitStack
    with tile.TileContext(nc) as tc, ExitStack() as stk:
        cpool = stk.enter_context(tc.tile_pool(name="consts", bufs=1))
        dram = stk.enter_context(tc.tile_pool(name="dram", bufs=1, space="DRAM"))
        ppool = stk.enter_context(tc.tile_pool(name="psum", bufs=1, space="PSUM"))
        spool = stk.enter_context(tc.tile_pool(name="small", bufs=1))

        iota128 = cpool.tile([P, 128], bf16)
        iota32 = cpool.tile([P, NHI], bf16)
        nc.sync.dma_start(iota128[:], io128_d.ap()[None, :].to_broadcast([P, 128]))
        nc.sync.dma_start(iota32[:], io32_d.ap()[None, :].to_broadcast([P, NHI]))

        # resident per-voxel tensors (kept in SBUF across all passes)
        xyz = cpool.tile([P, V, 3], bf16)
        lo = cpool.tile([P, V], bf16)
        hi = cpool.tile([P, V], bf16)
        nc.sync.dma_start(xyz[:], xyz_d.ap())
        nc.sync.dma_start(lo[:], lo_d.ap())
        nc.sync.dma_start(hi[:], hi_d.ap())
        lo32 = cpool.tile([P, V], f32)
        nc.vector.tensor_copy(out=lo32[:], in_=lo[:])

        # ---------------- pass 1: moment segment-sums ----------------
        no_mm = "nomm" in stop_after
        no_mf = "nomf" in stop_after
        no_oh = "nooh" in stop_after
        ps1 = ppool.tile([P, NHI * F], f32, space="PSUM")
        with tc.tile_pool(name="p1", bufs=2) as p1, \
             tc.tile_pool(name="p1oh", bufs=4) as p1oh:
            for c in range(NCH):
                sl = slice(c * G, (c + 1) * G)
                feat = p1.tile([P, G, F], bf16, tag="feat")
                nc.vector.memset(feat[:, :, 0:1], 1.0)
                nc.vector.tensor_copy(out=feat[:, :, 1:4], in_=xyz[:, sl, :])
                nc.vector.tensor_tensor(
                    out=feat[:, :, 4:7],
                    in0=xyz[:, sl, 0:1].to_broadcast([P, G, 3]),
                    in1=xyz[:, sl, 0:3], op=AO.mult)
                nc.vector.tensor_tensor(
                    out=feat[:, :, 7:9],
                    in0=xyz[:, sl, 1:2].to_broadcast([P, G, 2]),
                    in1=xyz[:, sl, 1:3], op=AO.mult)
                nc.vector.tensor_tensor(
                    out=feat[:, :, 9:10],
                    in0=xyz[:, sl, 2:3], in1=xyz[:, sl, 2:3], op=AO.mult)

                mask = p1.tile([P, G, NHI], bf16, tag="mask")
                nc.vector.tensor_tensor(
                    out=mask[:],
                    in0=hi[:, sl].unsqueeze(2).to_broadcast([P, G, NHI]),
                    in1=iota32[:].unsqueeze(1).to_broadcast([P, G, NHI]),
                    op=AO.is_equal)

                mf = p1.tile([P, G, NHI, F], bf16, tag="mf")
                if no_mf:
                    nc.vector.memset(mf[:], 0.5)
                else:
                    # split the 320-wide product between DVE and Pool
                    nc.vector.tensor_tensor(
                        out=mf[:, :, 0:mf_dve, :],
                        in0=mask[:, :, 0:mf_dve].unsqueeze(3).to_broadcast(
                            [P, G, mf_dve, F]),
                        in1=feat[:].unsqueeze(2).to_broadcast(
                            [P, G, mf_dve, F]),
                        op=AO.mult)
                    nc.gpsimd.tensor_tensor(
                        out=mf[:, :, mf_dve:NHI, :],
                        in0=mask[:, :, mf_dve:NHI].unsqueeze(3).to_broadcast(
                            [P, G, NHI - mf_dve, F]),
                        in1=feat[:].unsqueeze(2).to_broadcast(
                            [P, G, NHI - mf_dve, F]),
                        op=AO.mult)

                if not no_mm:
                    for t in range(G):
                        if no_oh:
                            oh = iota128
                        else:
                            oh = p1oh.tile([P, 128], bf16, tag="oh")
                            nc.vector.tensor_scalar(
                                out=oh[:], in0=iota128[:],
                                scalar1=lo32[:, c * G + t:c * G + t + 1],
                                scalar2=None, op0=AO.is_equal)
                        nc.tensor.matmul(
                            out=ps1[:], lhsT=oh[:],
                            rhs=mf[:, t].rearrange("p a b -> p (a b)"),
                            start=(c == 0 and t == 0),
                            stop=(c == NCH - 1 and t == G - 1))

        # ---------------- AllReduce moments ----------------
        S = spool.tile([P, NHI, F], f32)
        if no_mm:
            nc.vector.memset(S[:], 1.0)
        else:
            nc.vector.tensor_copy(out=S[:].rearrange("p a b -> p (a b)"),
                                  in_=ps1[:])
        cc1_in = dram.tile([P, NHI * F], f32)
        cc1_out = dram.tile([P, NHI * F], f32, addr_space="Shared")
        nc.sync.dma_start(cc1_in[:], S[:].rearrange("p a b -> p (a b)"))
        nc.gpsimd.collective_compute(
            "AllReduce", AO.add, replica_groups=groups,
            ins=[cc1_in[:]], outs=[cc1_out[:]])
        nc.sync.dma_start(S[:].rearrange("p a b -> p (a b)"), cc1_out[:])

        # ---------------- phase C: per-cluster eigensolve ----------------
        def new(name):
            return spool.tile([P, NHI], f32, name=name)

        def tt(out, a, b, op):
            nc.vector.tensor_tensor(out=out[:], in0=a[:], in1=b[:], op=op)

        def ts_(out, a, s1, op, s2=None, op1=None):
            kw = {}
            if op1 is not None:
                kw = dict(op1=op1)
            nc.vector.tensor_scalar(out=out[:], in0=a[:], scalar1=s1, scalar2=s2,
                                    op0=op, **kw)

        dtmp1 = new("dtmp1"); dtmp2 = new("dtmp2")

        def recip(out, den):
            nc.vector.reciprocal(out=dtmp1[:], in_=den[:])
            tt(dtmp2, den, dtmp1, AO.mult)
            ts_(dtmp2, dtmp2, -1.0, AO.mult, s2=2.0, op1=AO.add)
            tt(out, dtmp1, dtmp2, AO.mult)

        cnt = new("cnt"); safe = new("safe")
        nc.vector.tensor_copy(out=cnt[:], in_=S[:, :, 0])
        ts_(safe, cnt, 1.0, AO.max)
        rsafe = new("rsafe")
        recip(rsafe, safe)

        ctr = [new(f"ctr{k}") for k in range(3)]
        for k in range(3):
            nc.vector.tensor_tensor(out=ctr[k][:], in0=S[:, :, 1 + k],
                                    in1=rsafe[:], op=AO.mult)

        cc = [new(f"cc{k}") for k in range(3)]
        for k in range(3):
            tt(cc[k], cnt, ctr[k], AO.mult)
        pairs = [(0, 0), (0, 1), (0, 2), (1, 1), (1, 2), (2, 2)]
        cov = [new(f"cov{k}") for k in range(6)]
        tmp = new("tmp"); tmp2 = new("tmp2"); tmp3 = new("tmp3")
        for k, (i, j) in enumerate(pairs):
            tt(tmp, ctr[i], cc[j], AO.mult)
            nc.vector.tensor_tensor(out=cov[k][:], in0=S[:, :, 4 + k],
                                    in1=tmp[:], op=AO.subtract)
        XX, XY, XZ, YY, YZ, ZZ = range(6)

        qm = new("qm")
        tt(qm, cov[XX], cov[YY], AO.add)
        tt(qm, qm, cov[ZZ], AO.add)
        ts_(qm, qm, 1.0 / 3.0, AO.mult)
        aq = [new(f"aq{k}") for k in range(3)]
        tt(aq[0], cov[XX], qm, AO.subtract)
        tt(aq[1], cov[YY], qm, AO.subtract)
        tt(aq[2], cov[ZZ], qm, AO.subtract)
        p2v = new("p2v")
        tt(p2v, aq[0], aq[0], AO.mult)
        tt(tmp, aq[1], aq[1], AO.mult); tt(p2v, p2v, tmp, AO.add)
        tt(tmp, aq[2], aq[2], AO.mult); tt(p2v, p2v, tmp, AO.add)
        for k in (XY, XZ, YZ):
            tt(tmp, cov[k], cov[k], AO.mult)
            ts_(tmp, tmp, 2.0, AO.mult)
            tt(p2v, p2v, tmp, AO.add)
        pp = new("pp")
        ts_(tmp, p2v, 1.0 / 6.0, AO.mult)
        nc.scalar.sqrt(pp[:], tmp[:])
        psafe = new("psafe"); rpsafe = new("rpsafe")
        ts_(psafe, pp, TINY, AO.max)
        recip(rpsafe, psafe)

        bn = [new(f"bn{k}") for k in range(6)]
        for k, (i, j) in enumerate(pairs):
            src = cov[k]
            if i == j:
                src = aq[{0: 0, 3: 1, 5: 2}[k]]
            nc.vector.tensor_tensor(out=bn[k][:], in0=src[:], in1=rpsafe[:],
                                    op=AO.mult)
        det = new("det")
        tt(tmp, bn[YY], bn[ZZ], AO.mult)
        tt(tmp2, bn[YZ], bn[YZ], AO.mult)
        tt(tmp, tmp, tmp2, AO.subtract)
        tt(det, bn[XX], tmp, AO.mult)
        tt(tmp, bn[XY], bn[ZZ], AO.mult)
        tt(tmp2, bn[YZ], bn[XZ], AO.mult)
        tt(tmp, tmp, tmp2, AO.subtract)
        tt(tmp, tmp, bn[XY], AO.mult)
        tt(det, det, tmp, AO.subtract)
        tt(tmp, bn[XY], bn[YZ], AO.mult)
        tt(tmp2, bn[YY], bn[XZ], AO.mult)
        tt(tmp, tmp, tmp2, AO.subtract)
        tt(tmp, tmp, bn[XZ], AO.mult)
        tt(det, det, tmp, AO.add)
        r = new("r")
        ts_(r, det, 0.5, AO.mult, s2=1.0, op1=AO.min)
        ts_(r, r, -1.0, AO.max)

        omr = new("omr"); opr = new("opr"); sig = new("sig"); absr = new("absr")
        ts_(omr, r, -1.0, AO.mult, s2=1.0, op1=AO.add)
        ts_(opr, r, 1.0, AO.add)
        tt(tmp, omr, opr, AO.mult)
        nc.scalar.sqrt(sig[:], tmp[:])
        nc.scalar.activation(absr[:], r[:], AF.Abs)
        ts_(tmp, absr, 1.0, AO.add)
        recip(tmp3, tmp)
        tt(tmp2, sig, tmp3, AO.mult)
        phi = new("phi"); rneg = new("rneg")
        nc.scalar.activation(phi[:], tmp2[:], AF.Arctan)
        ts_(phi, phi, 2.0 / 3.0, AO.mult)
        ts_(rneg, r, 0.0, AO.is_lt)
        ts_(tmp, rneg, -2.0, AO.mult, s2=1.0, op1=AO.add)
        tt(phi, phi, tmp, AO.mult)
        ts_(tmp, rneg, float(np.pi / 3.0), AO.mult)
        tt(phi, phi, tmp, AO.add)

        w0 = new("w0"); w1 = new("w1"); w2 = new("w2")
        ts_(tmp, phi, -1.0, AO.mult, s2=float(np.pi / 2), op1=AO.add)
        nc.scalar.activation(tmp2[:], tmp[:], AF.Sin)
        tt(tmp2, tmp2, pp, AO.mult)
        ts_(tmp2, tmp2, 2.0, AO.mult)
        tt(w2, qm, tmp2, AO.add)
        ts_(tmp, phi, -1.0, AO.mult, s2=float(-np.pi / 6), op1=AO.add)
        nc.scalar.activation(tmp2[:], tmp[:], AF.Sin)
        tt(tmp2, tmp2, pp, AO.mult)
        ts_(tmp2, tmp2, 2.0, AO.mult)
        tt(w0, qm, tmp2, AO.add)
        ts_(tmp, qm, 3.0, AO.mult)
        tt(tmp, tmp, w0, AO.subtract)
        tt(w1, tmp, w2, AO.subtract)

        w2z = new("w2z"); dirwt = new("dirwt")
        ts_(w2z, w2, 0.0, AO.is_equal)
        ts_(tmp, w2, TINY, AO.max)
        recip(tmp3, tmp)
        tt(tmp2, w1, tmp3, AO.mult)
        ts_(tmp2, tmp2, -1.0, AO.mult, s2=1.0, op1=AO.add)
        ts_(tmp, w2z, -1.0, AO.mult, s2=1.0, op1=AO.add)
        tt(dirwt, tmp2, tmp, AO.mult)

        denb = new("denb"); rdenb = new("rdenb")
        tt(denb, w2, w2z, AO.add)
        recip(rdenb, denb)
        Bk = [new(f"B{k}") for k in range(6)]
        for k in range(6):
            tt(Bk[k], cov[k], rdenb, AO.mult)

        d0 = [new(f"d0{k}") for k in range(3)]
        d1 = [new(f"d1{k}") for k in range(3)]
        for k, dk in enumerate((XX, YY, ZZ)):
            tt(d0[k], cov[dk], w0, AO.subtract)
            tt(d1[k], cov[dk], w1, AO.subtract)
        A0 = [[d0[0], cov[XY], cov[XZ]],
              [cov[XY], d0[1], cov[YZ]],
              [cov[XZ], cov[YZ], d0[2]]]
        A1 = [[d1[0], cov[XY], cov[XZ]],
              [cov[XY], d1[1], cov[YZ]],
              [cov[XZ], cov[YZ], d1[2]]]
        M = [[new(f"M{i}{j}") for j in range(3)] for i in range(3)]
        for i in range(3):
            for j in range(3):
                tt(M[i][j], A0[i][0], A1[0][j], AO.mult)
                tt(tmp, A0[i][1], A1[1][j], AO.mult)
                tt(M[i][j], M[i][j], tmp, AO.add)
                tt(tmp, A0[i][2], A1[2][j], AO.mult)
                tt(M[i][j], M[i][j], tmp, AO.add)
        nrm = [new(f"nrm{j}") for j in range(3)]
        for j in range(3):
            tt(nrm[j], M[0][j], M[0][j], AO.mult)
            tt(tmp, M[1][j], M[1][j], AO.mult)
            tt(nrm[j], nrm[j], tmp, AO.add)
            tt(tmp, M[2][j], M[2][j], AO.mult)
            tt(nrm[j], nrm[j], tmp, AO.add)
        vbest = [new(f"vb{i}") for i in range(3)]
        nbest = new("nbest")
        mask = spool.tile([P, NHI], mybir.dt.int32, name="mask")
        tt(mask, nrm[1], nrm[0], AO.is_gt)
        for i in range(3):
            nc.vector.select(vbest[i][:], mask[:], M[i][1][:], M[i][0][:])
        nc.vector.select(nbest[:], mask[:], nrm[1][:], nrm[0][:])
        tt(mask, nrm[2], nbest, AO.is_gt)
        for i in range(3):
            nc.vector.select(vbest[i][:], mask[:], M[i][2][:], vbest[i][:])
        nc.vector.select(nbest[:], mask[:], nrm[2][:], nbest[:])
        vhat = [new(f"vh{i}") for i in range(3)]
        nc.scalar.sqrt(tmp[:], nbest[:])
        ts_(tmp, tmp, TINY, AO.max)
        recip(tmp3, tmp)
        for i in range(3):
            tt(vhat[i], vbest[i], tmp3, AO.mult)

        small = new("small"); notsmall = new("notsmall")
        ts_(small, cnt, 2.0, AO.is_lt)
        ts_(notsmall, small, -1.0, AO.mult, s2=1.0, op1=AO.add)

        if stop_after != "pc":
            # gather table rows (bf16): [cx,cy,cz, vx,vy,vz, 0,0]
            table_d = dram.tile([C, 8], bf16)
            Gt = spool.tile([P, NHI, 8], bf16)
            nc.vector.memset(Gt[:, :, 6:8], 0.0)
            for k in range(3):
                nc.vector.tensor_copy(out=Gt[:, :, k], in_=ctr[k][:])
                nc.vector.tensor_copy(out=Gt[:, :, 3 + k], in_=vhat[k][:])
            # DRAM row c = hi*128 + lo  ->  partitions are lo, free dim hi
            nc.sync.dma_start(
                table_d[:].rearrange("(a l) e -> l a e", l=P), Gt[:])

            # ---------------- pass 2+3: sc segment-sum ----------------
            ps_sc = ppool.tile([P, NHI], f32, space="PSUM")
            with tc.tile_pool(name="p2", bufs=2) as p2, \
                 tc.tile_pool(name="p2oh", bufs=4) as p2oh:
                for c in range(NCH):
                    sl = slice(c * G, (c + 1) * G)
                    idg = p2.tile([P, G], i32, tag="idg")
                    nc.sync.dma_start(idg[:], idg_d.ap()[:, sl])
                    gat = p2.tile([P, G, 8], bf16, tag="gat")
                    if "nogather" in stop_after:
                        nc.vector.memset(gat[:], 0.125)
                    else:
                        nc.gpsimd.indirect_dma_start(
                            out=gat[:], out_offset=None,
                            in_=table_d[:],
                            in_offset=bass.IndirectOffsetOnAxis(
                                ap=idg[:], axis=0))

                    xc = p2.tile([P, G, 3], bf16, tag="xc")
                    nc.vector.tensor_tensor(out=xc[:], in0=xyz[:, sl, :],
                                            in1=gat[:, :, 0:3], op=AO.subtract)
                    prod = p2.tile([P, G, 3], bf16, tag="prod")
                    nc.vector.tensor_tensor(out=prod[:], in0=xc[:],
                                            in1=gat[:, :, 3:6], op=AO.mult)
                    x0 = p2.tile([P, G], f32, tag="x0")
                    nc.vector.tensor_reduce(out=x0[:], in_=prod[:],
                                            axis=mybir.AxisListType.X, op=AO.add)
                    nc.vector.tensor_tensor(out=prod[:], in0=xc[:], in1=xc[:],
                                            op=AO.mult)
                    nsq = p2.tile([P, G], f32, tag="nsq")
                    nc.vector.tensor_reduce(out=nsq[:], in_=prod[:],
                                            axis=mybir.AxisListType.X, op=AO.add)
                    val = p2.tile([P, G], f32, tag="val")
                    nc.vector.tensor_tensor(out=val[:], in0=x0[:], in1=x0[:],
                                            op=AO.mult)
                    nc.vector.tensor_tensor(out=val[:], in0=nsq[:], in1=val[:],
                                            op=AO.subtract)
                    nc.vector.tensor_scalar(out=val[:], in0=val[:], scalar1=0.0,
                                            scalar2=None, op0=AO.max)
                    nc.scalar.sqrt(val[:], val[:])
                    valb = p2.tile([P, G], bf16, tag="valb")
                    nc.vector.tensor_tensor(out=valb[:], in0=val[:], in1=x0[:],
                                            op=AO.mult)

                    mask2 = p2.tile([P, G, NHI], bf16, tag="mask2")
                    nc.vector.tensor_tensor(
                        out=mask2[:],
                        in0=hi[:, sl].unsqueeze(2).to_broadcast([P, G, NHI]),
                        in1=iota32[:].unsqueeze(1).to_broadcast([P, G, NHI]),
                        op=AO.is_equal)
                    mval = p2.tile([P, G, NHI], bf16, tag="mval")
                    nc.vector.tensor_tensor(
                        out=mval[:], in0=mask2[:],
                        in1=valb[:].unsqueeze(2).to_broadcast([P, G, NHI]),
                        op=AO.mult)

                    for t in range(G):
                        oh = p2oh.tile([P, 128], bf16, tag="oh3")
                        nc.vector.tensor_scalar(
                            out=oh[:], in0=iota128[:],
                            scalar1=lo32[:, c * G + t:c * G + t + 1],
                            scalar2=None, op0=AO.is_equal)
                        nc.tensor.matmul(
                            out=ps_sc[:], lhsT=oh[:], rhs=mval[:, t],
                            start=(c == 0 and t == 0),
                            stop=(c == NCH - 1 and t == G - 1))

            scl = spool.tile([P, NHI], f32)
            nc.vector.tensor_copy(out=scl[:], in_=ps_sc[:])
            sc = spool.tile([P, NHI], f32)
            if "nocc2" in stop_after:
                nc.vector.tensor_copy(out=sc[:], in_=scl[:])
            else:
                cc2_in = dram.tile([P, NHI], f32)
                cc2_out = dram.tile([P, NHI], f32, addr_space="Shared")
                nc.sync.dma_start(cc2_in[:], scl[:])
                nc.gpsimd.collective_compute(
                    "AllReduce", AO.add, replica_groups=groups,
                    ins=[cc2_in[:]], outs=[cc2_out[:]])
                nc.sync.dma_start(sc[:], cc2_out[:])

            # ---------------- phase E: assemble output ----------------
            flip = new("flip"); scale = new("scale")
            ts_(flip, sc, 0.0, AO.is_lt)
            ts_(flip, flip, -2.0, AO.mult, s2=1.0, op1=AO.add)
            tt(scale, dirwt, flip, AO.mult)
            tt(scale, scale, notsmall, AO.mult)

            OUT = spool.tile([P, NHI, 16], f32)
            for k in range(3):
                nc.vector.tensor_copy(out=OUT[:, :, k], in_=ctr[k][:])
            bidx = [XX, XY, XZ, XY, YY, YZ, XZ, YZ, ZZ]
            for k in range(9):
                tt(tmp, Bk[bidx[k]], notsmall, AO.mult)
                nc.vector.tensor_copy(out=OUT[:, :, 3 + k], in_=tmp[:])
            for k in range(3):
                tt(tmp, vhat[k], scale, AO.mult)
                nc.vector.tensor_copy(out=OUT[:, :, 12 + k], in_=tmp[:])
            nc.vector.tensor_copy(out=OUT[:, :, 15], in_=cnt[:])
            nc.sync.dma_start(
                out_d.ap().rearrange("(a l) e -> l a e", l=P), OUT[:])
        else:
            OUTD = spool.tile([P, NHI, 16], f32)
            nc.vector.memset(OUTD[:], 0.0)
            for k in range(3):
                nc.vector.tensor_copy(out=OUTD[:, :, k], in_=ctr[k][:])
                nc.vector.tensor_copy(out=OUTD[:, :, 12 + k], in_=vhat[k][:])
            nc.vector.tensor_copy(out=OUTD[:, :, 15], in_=cnt[:])
            nc.sync.dma_start(
                out_d.ap().rearrange("(a l) e -> l a e", l=P), OUTD[:])
    nc.compile()
    return nc


_CACHE = {}


def _get(V, G, n_cores, stop_after="full", mf_dve=12):
    key = (V, G, n_cores, stop_after, mf_dve)
    if key not in _CACHE:
        _CACHE[key] = build(V, G, n_cores, stop_after, mf_dve)
    return _CACHE[key]


def run(data, clust_ids, V=3968, G=64, n_cores=NCORES,
        stop_after="full", mf_dve=12, trace=False):
    """data [N,>=3] f32, clust_ids [N] int -> [4096, 16] f32."""
    n = data.shape[0]
    per = n // n_cores
    assert per * n_cores == n and per <= P * V
    xyz = np.ascontiguousarray(np.asarray(data[:, :3], dtype=np.float32))
    ids = np.asarray(clust_ids).astype(np.int32)
    io128 = np.arange(128, dtype=np.float32)
    io32 = np.arange(NHI, dtype=np.float32)
    import ml_dtypes
    to_bf = lambda a: a.astype(ml_dtypes.bfloat16)
    in_maps = []
    for k in range(n_cores):
        xp = np.zeros((P * V, 3), np.float32)
        xp[:per] = xyz[k * per:(k + 1) * per]
        ip = np.full((P * V,), C, np.int32)
        ip[:per] = ids[k * per:(k + 1) * per]
        lo_ = (ip & 127).astype(np.float32)
        hi_ = (ip >> 7).astype(np.float32)   # pad voxels get hi=32 -> masked out
        idg = np.minimum(ip, C - 1).astype(np.int32)
        in_maps.append({"xyz": to_bf(xp).reshape(P, V, 3),
                        "lo": to_bf(lo_).reshape(P, V),
                        "hi": to_bf(hi_).reshape(P, V),
                        "idg": idg.reshape(P, V),
                        "iota128": to_bf(io128), "iota32": to_bf(io32)})
    nc = _get(V, G, n_cores, stop_after, mf_dve)
    res = bass_utils.run_bass_kernel_spmd(nc, in_maps,
                                          core_ids=list(range(n_cores)),
                                          trace=trace)
    return res.results[0]["out"], res


def kernel(data, clust_ids, num_clusters=C):
    out, _ = run(np.asarray(data), np.asarray(clust_ids))
    return out
